# revision 6
# baseline (speedup 1.0000x reference)
"""Trainium2 Bass kernel for CloudGNN (GCN -> GAT -> SAGE -> heads).

Strategy (graph/data parallel over 8 NeuronCores):
  - Nodes padded to N_pad = 8 * nodes_per_core, partitioned contiguously;
    each core owns W windows of 128 target nodes.
  - Edges assigned by target (col) window. Per (core, window) edges split by
    source row < 32768 (dma_gather idx is int16) into A/B groups, each padded
    to a multiple of 128 (an "edge tile"). Tile counts per window slot are
    global maxima over cores so one SPMD program fits every core.
  - Per edge tile [128 edges]: source payload rows fetched with dma_gather;
    a one-hot G[e, c] = (lcol[e] == c) built on-device (iota + is_equal);
    segment-sum = PE matmul accumulated in PSUM across the window's tiles.
  - GCN aggregates raw x (linearity: (sum w_e x_row) @ W1); dinv[row] folded
    into G, dinv[col] applied on output. Self-loops are ordinary edges.
  - GAT gathers hg_ext = [hg (256) | a_src (4) | pad] rows; a_dst[col] comes
    from a col-indexed core-local gather; softmax computed unnormalized
    (exp without max-subtraction; numerator + denominator are segment sums
    in the same PSUM matmul; divide densely at the end).
  - SAGE aggregates h2 rows over the no-self-loop edge set; 1/deg0 and
    Wl/Wr/bl applied densely per window.
  - Two AllGathers: h1^T blocks (for dense hg recompute) and h2 (SAGE table).
"""

import sys

sys.path.insert(0, "/opt/trn_rl_repo")

import numpy as np

import concourse.bass as bass
import concourse.bacc as bacc
import concourse.tile as tile
from concourse import mybir
from concourse.bass_utils import run_bass_kernel_spmd

F32 = mybir.dt.float32
I32 = mybir.dt.int32
I16 = mybir.dt.int16
AOT = mybir.AluOpType
ACTF = mybir.ActivationFunctionType

P = 128
NCORES = 8
TH = 32768  # int16 gather index limit


def _bc(ap, count):
    """Append a [0, count] broadcast dim to an AP."""
    return bass.AP(ap.tensor, ap.offset, list(ap.ap) + [[0, count]])


def _split_last(ap, h, f):
    """Replace last dim [1, h*f] of an AP with [f, h], [1, f]."""
    pat = list(ap.ap)
    assert pat[-1][0] == 1 and pat[-1][1] == h * f
    return bass.AP(ap.tensor, ap.offset, pat[:-1] + [[f, h], [1, f]])


# ----------------------------------------------------------------------------
# Host preprocessing
# ----------------------------------------------------------------------------

def _wrap_idx(idx_flat):
    """int16 gather idx layout: idx i at [i%16, i//16], replicated x8 -> 128 rows."""
    n = idx_flat.shape[0]
    assert n % 128 == 0
    w = idx_flat.reshape(n // 16, 16).T.astype(np.int16)
    return np.tile(w, (8, 1))


def _build_edge_set(row, col, weight, W, npc, table_rows):
    row = row.astype(np.int64)
    col = col.astype(np.int64)
    w_abs = col // P
    core = w_abs // W
    slot = w_abs % W
    half = (row >= TH).astype(np.int64)
    key = (core * W + slot) * 2 + half
    order = np.argsort(key, kind="stable")
    row_s, col_s = row[order], col[order]
    wt_s = weight[order] if weight is not None else None

    counts = np.bincount(key, minlength=NCORES * W * 2).reshape(NCORES, W, 2)
    tiles = -(-counts // P)
    TA = tiles[:, :, 0].max(axis=0)
    TB = tiles[:, :, 1].max(axis=0)
    sumTA, sumTB = int(TA.sum()), int(TB.sum())
    sumT = sumTA + sumTB

    starts = np.zeros(NCORES * W * 2 + 1, dtype=np.int64)
    np.cumsum(counts.reshape(-1), out=starts[1:])

    per_core = []
    for c in range(NCORES):
        idxA = np.zeros(max(1, sumTA) * P, dtype=np.int64)
        idxB = np.zeros(max(1, sumTB) * P, dtype=np.int64)
        lcol = np.full(sumT * P, -1.0, dtype=np.float32)
        dinw = np.zeros(sumT * P, dtype=np.float32)
        lcolc = np.zeros(sumT * P, dtype=np.int64)
        offA = offB = offT = 0
        for j in range(W):
            for h in (0, 1):
                Tn = int(TA[j] if h == 0 else TB[j])
                k = (c * W + j) * 2 + h
                s, e = starts[k], starts[k + 1]
                n = int(e - s)
                if h == 0:
                    idxA[offA:offA + n] = row_s[s:e]
                    offA += Tn * P
                else:
                    idxB[offB:offB + n] = row_s[s:e] - TH
                    offB += Tn * P
                lcol[offT:offT + n] = (col_s[s:e] % P).astype(np.float32)
                lcolc[offT:offT + n] = col_s[s:e] - c * npc
                if wt_s is not None:
                    dinw[offT:offT + n] = wt_s[s:e]
                else:
                    dinw[offT:offT + n] = 1.0
                offT += Tn * P
        assert idxA.max(initial=0) < table_rows
        assert idxB.max(initial=0) < max(1, table_rows - TH)
        assert lcolc.max(initial=0) < npc
        per_core.append(dict(
            idxA=_wrap_idx(idxA),
            idxB=_wrap_idx(idxB),
            colidx=_wrap_idx(lcolc) if sumT else np.zeros((128, 1), np.int16),
            lcol=np.ascontiguousarray(lcol.reshape(sumT, P).T),
            dinw=np.ascontiguousarray(dinw.reshape(sumT, P).T),
        ))
    return dict(TA=TA.tolist(), TB=TB.tolist(), per_core=per_core,
                sumTA=sumTA, sumTB=sumTB, sumT=sumT)


# ----------------------------------------------------------------------------
# Device program
# ----------------------------------------------------------------------------

def build_program(N, F_IN, H, HEADS, W, npc, e1, e0, n_cores=NCORES,
                  phases="B1CD2E"):
    N_pad = npc * n_cores
    HH = HEADS * H                       # 256
    EXT = HH + 64                        # hg_ext row: hg | a_src | pad = 320
    TA1, TB1 = e1["TA"], e1["TB"]
    TA0, TB0 = e0["TA"], e0["TB"]
    sT1, sT0 = e1["sumT"], e0["sumT"]
    Tmax1 = max(TA1[j] + TB1[j] for j in range(W))
    Tmax0 = max((TA0[j] + TB0[j] for j in range(W)), default=1)
    TmaxH = max(max(TA1), max(TB1), 1)  # max tiles in any single gather

    nc = bacc.Bacc("TRN2", target_bir_lowering=False, debug=False,
                   num_devices=n_cores)

    x_in = nc.dram_tensor("x", [N, F_IN], F32, kind="ExternalInput")
    W1_in = nc.dram_tensor("W1", [F_IN, H], F32, kind="ExternalInput")
    Wg_in = nc.dram_tensor("Wg", [H, HH], F32, kind="ExternalInput")
    Wsrc_in = nc.dram_tensor("Wsrc", [H, HEADS], F32, kind="ExternalInput")
    Wdst_in = nc.dram_tensor("Wdst", [H, HEADS], F32, kind="ExternalInput")
    Wl_in = nc.dram_tensor("Wl", [H, H], F32, kind="ExternalInput")
    Wr_in = nc.dram_tensor("Wr", [H, H], F32, kind="ExternalInput")
    ar1w_in = nc.dram_tensor("ar1w", [H, 64], F32, kind="ExternalInput")
    ar2w_in = nc.dram_tensor("ar2w", [64, 2], F32, kind="ExternalInput")
    b1r_in = nc.dram_tensor("b1r", [128, H], F32, kind="ExternalInput")
    bgr_in = nc.dram_tensor("bgr", [128, H], F32, kind="ExternalInput")
    blr_in = nc.dram_tensor("blr", [128, H], F32, kind="ExternalInput")
    ar1br_in = nc.dram_tensor("ar1br", [128, 64], F32, kind="ExternalInput")
    ar2br_in = nc.dram_tensor("ar2br", [128, 2], F32, kind="ExternalInput")
    dinvo_in = nc.dram_tensor("dinv_own", [128, W], F32, kind="ExternalInput")
    inv0o_in = nc.dram_tensor("inv0_own", [128, W], F32, kind="ExternalInput")
    e1_idxA_in = nc.dram_tensor("e1_idxA", [128, max(1, e1["sumTA"]) * 8], I16, kind="ExternalInput")
    e1_idxB_in = nc.dram_tensor("e1_idxB", [128, max(1, e1["sumTB"]) * 8], I16, kind="ExternalInput")
    e1_col_in = nc.dram_tensor("e1_col", [128, max(1, sT1) * 8], I16, kind="ExternalInput")
    e1_lcol_in = nc.dram_tensor("e1_lcol", [128, sT1], F32, kind="ExternalInput")
    e1_dinw_in = nc.dram_tensor("e1_dinw", [128, sT1], F32, kind="ExternalInput")
    e0_idxA_in = nc.dram_tensor("e0_idxA", [128, max(1, e0["sumTA"]) * 8], I16, kind="ExternalInput")
    e0_idxB_in = nc.dram_tensor("e0_idxB", [128, max(1, e0["sumTB"]) * 8], I16, kind="ExternalInput")
    e0_lcol_in = nc.dram_tensor("e0_lcol", [128, max(1, sT0)], F32, kind="ExternalInput")

    emb_out = nc.dram_tensor("emb_own", [npc, H], F32, kind="ExternalOutput")
    outp_out = nc.dram_tensor("outp_own", [npc, 2], F32, kind="ExternalOutput")

    with tile.TileContext(nc) as tc:
        with tc.tile_pool(name="dram", bufs=1, space="DRAM") as dram, \
             tc.tile_pool(name="const", bufs=1) as cpool, \
             tc.tile_pool(name="persist", bufs=1) as ppool:

            hg_ext = dram.tile([N_pad, EXT], F32)
            adst_tab = dram.tile([npc, 64], F32)
            h1T_shard = dram.tile([H, npc], F32)
            h1T_blocks = dram.tile([H * n_cores, npc], F32)
            h2_shard = dram.tile([npc, H], F32)
            h2_full = dram.tile([N_pad, H], F32)

            # ---- constants ----
            iota_i = cpool.tile([128, Tmax1 * 128], I32, tag="iota_i")
            nc.gpsimd.iota(iota_i[:], pattern=[[0, Tmax1], [1, 128]], base=0,
                           channel_multiplier=0)
            iota_rep = cpool.tile([128, Tmax1 * 128], F32, tag="iota_rep")
            nc.vector.tensor_copy(iota_rep[:], iota_i[:])
            pidx_i = cpool.tile([128, 1], I32, tag="pidx_i")
            nc.gpsimd.iota(pidx_i[:], pattern=[[0, 1]], base=0, channel_multiplier=1)
            pidx_f = cpool.tile([128, 1], F32, tag="pidx_f")
            nc.vector.tensor_copy(pidx_f[:], pidx_i[:])
            ident = cpool.tile([128, 128], F32, tag="ident")
            nc.vector.tensor_scalar(ident[:], iota_rep[:, 0:128], pidx_f[:], None,
                                    AOT.is_equal)

            def load_const(name, src, shape):
                t = cpool.tile(list(shape), F32, tag=name)
                nc.sync.dma_start(t[:], src[:])
                return t

            W1_sb = load_const("W1c", W1_in, (F_IN, H))
            Wg_sb = load_const("Wgc", Wg_in, (H, HH))
            Wsrc_sb = load_const("Wsrcc", Wsrc_in, (H, HEADS))
            Wdst_sb = load_const("Wdstc", Wdst_in, (H, HEADS))
            Wl_sb = load_const("Wlc", Wl_in, (H, H))
            Wr_sb = load_const("Wrc", Wr_in, (H, H))
            ar1w_sb = load_const("ar1wc", ar1w_in, (H, 64))
            ar2w_sb = load_const("ar2wc", ar2w_in, (64, 2))
            b1r_sb = load_const("b1rc", b1r_in, (128, H))
            bgr_sb = load_const("bgrc", bgr_in, (128, H))
            blr_sb = load_const("blrc", blr_in, (128, H))
            ar1br_sb = load_const("ar1brc", ar1br_in, (128, 64))
            ar2br_sb = load_const("ar2brc", ar2br_in, (128, 2))
            dinvo_sb = load_const("dinvoc", dinvo_in, (128, W))
            inv0o_sb = load_const("inv0oc", inv0o_in, (128, W))
            e1_lcol_sb = load_const("e1lcolc", e1_lcol_in, (128, sT1))
            e1_dinw_sb = load_const("e1dinwc", e1_dinw_in, (128, sT1))
            e0_lcol_sb = load_const("e0lcolc", e0_lcol_in, (128, max(1, sT0)))

            h1T_own = ppool.tile([H, npc], F32, tag="h1T_own")
            h2T_own = ppool.tile([H, npc], F32, tag="h2T_own")

            def build_G(pool, bufs_tag, lcol_sb, wt_sb, t_off, T):
                G = pool.tile([128, Tmax1 * 128], F32, tag=bufs_tag)
                Gv = G[:].rearrange("p (t c) -> p t c", c=128)[:, 0:T]
                i0 = iota_rep[:].rearrange("p (t c) -> p t c", c=128)[:, 0:T]
                l1 = _bc(lcol_sb[:, t_off:t_off + T], 128)
                nc.vector.tensor_tensor(Gv, i0, l1, AOT.is_equal)
                if wt_sb is not None:
                    w1 = _bc(wt_sb[:, t_off:t_off + T], 128)
                    nc.vector.tensor_tensor(Gv, Gv, w1, AOT.mult)
                return G

            def gather_window(pool, tag, idxA_in, idxB_in, tabA, tabB,
                              offA, offB, TAj, TBj, elem, tmax):
                """A+B gathers for one window -> payload tile [128, tmax*elem]."""
                pay = pool.tile([128, tmax * elem], F32, tag=tag)
                pay3 = pay[:].rearrange("p (t f) -> p t f", f=elem)
                for h, (Tn, idx_in, off_t, tab) in enumerate((
                    (TAj, idxA_in, offA, tabA),
                    (TBj, idxB_in, offB, tabB),
                )):
                    if Tn == 0:
                        continue
                    n = Tn * 128
                    it = pool.tile([128, TmaxH * 8], I16, tag=tag + "_idx")
                    nc.sync.dma_start(it[:, :n // 16],
                                      idx_in[:, off_t * 8: off_t * 8 + n // 16])
                    dst = pay3[:, 0:Tn] if h == 0 else pay3[:, TAj:TAj + Tn]
                    nc.gpsimd.dma_gather(dst, tab, it[:, :n // 16],
                                         num_idxs=n, num_idxs_reg=n,
                                         elem_size=elem, single_packet=False)
                return pay

            xB = x_in[TH:N, :] if N > TH else None
            hgB = hg_ext[:][TH:N_pad, :] if N_pad > TH else None
            h2B = h2_full[:][TH:N_pad, :] if N_pad > TH else None

            # ================= Phase B: GCN =================
            offA = offB = offT = 0
            if "B" in phases:
              with tc.tile_pool(name="gcn_sb", bufs=2) as gpool, \
                 tc.tile_pool(name="gcn_ps", bufs=2, space="PSUM") as gps, \
                 tc.tile_pool(name="gcn_ps2", bufs=1, space="PSUM") as gps2:
                for j in range(W):
                    TAj, TBj = TA1[j], TB1[j]
                    T = TAj + TBj
                    pay = gather_window(gpool, "gcn_pay", e1_idxA_in, e1_idxB_in,
                                        x_in[:], xB, offA, offB, TAj, TBj,
                                        F_IN, Tmax1)
                    pay3 = pay[:].rearrange("p (t f) -> p t f", f=F_IN)
                    G = build_G(gpool, "gcn_G", e1_lcol_sb, e1_dinw_sb, offT, T)
                    G3 = G[:].rearrange("p (t c) -> p t c", c=128)
                    aggT = gps.tile([128, 128], F32, tag="gcn_aggT")
                    for t in range(T):
                        nc.tensor.matmul(aggT[:], pay3[:, t], G3[:, t],
                                         start=(t == 0), stop=(t == T - 1))
                    aggT_sb = gpool.tile([128, 128], F32, tag="gcn_aggT_sb")
                    nc.vector.tensor_copy(aggT_sb[:], aggT[:])
                    h1_ps = gps2.tile([128, H], F32, tag="gcn_h1")
                    nc.tensor.matmul(h1_ps[:], aggT_sb[:], W1_sb[:],
                                     start=True, stop=True)
                    h1_sb = gpool.tile([128, H], F32, tag="gcn_h1_sb")
                    nc.vector.tensor_scalar_mul(h1_sb[:], h1_ps[:],
                                                dinvo_sb[:, j:j + 1])
                    nc.vector.tensor_tensor(h1_sb[:], h1_sb[:], b1r_sb[:], AOT.add)
                    nc.vector.tensor_scalar_max(h1_sb[:], h1_sb[:], 0.0)
                    tr_ps = gps2.tile([H, 128], F32, tag="gcn_tr")
                    nc.tensor.transpose(tr_ps[:], h1_sb[:], ident[:])
                    nc.vector.tensor_copy(h1T_own[:, j * 128:(j + 1) * 128], tr_ps[:])
                    adst_ps = gps2.tile([128, HEADS], F32, tag="gcn_adst")
                    nc.tensor.matmul(adst_ps[:], h1T_own[:, j * 128:(j + 1) * 128],
                                     Wdst_sb[:], start=True, stop=True)
                    adst_sb = gpool.tile([128, 64], F32, tag="gcn_adst_sb")
                    nc.vector.memset(adst_sb[:], 0.0)
                    nc.vector.tensor_copy(adst_sb[:, 0:HEADS], adst_ps[:])
                    nc.sync.dma_start(
                        adst_tab[:].rearrange("(w p) f -> w p f", p=128)[j],
                        adst_sb[:])
                    offA += TAj
                    offB += TBj
                    offT += T
                nc.sync.dma_start(h1T_shard[:], h1T_own[:])

            if "1" in phases:
                nc.gpsimd.collective_compute(
                    "AllGather", AOT.bypass,
                    replica_groups=[list(range(n_cores))],
                    ins=[h1T_shard.opt()], outs=[h1T_blocks.opt()])

            # ================= Phase C: hg_ext table =================
            if "C" in phases:
              with tc.tile_pool(name="hg_sb", bufs=2) as hpool, \
                 tc.tile_pool(name="hg_ps", bufs=2, space="PSUM") as hps:
                for blk in range(n_cores):
                    h1T_blk = hpool.tile([H, npc], F32, tag="h1T_blk")
                    nc.sync.dma_start(h1T_blk[:],
                                      h1T_blocks[:][blk * H:(blk + 1) * H, :])
                    for j in range(W):
                        lhsT = h1T_blk[:, j * 128:(j + 1) * 128]
                        hg_ps = hps.tile([128, HH], F32, tag="hg_ps")
                        nc.tensor.matmul(hg_ps[:], lhsT, Wg_sb[:],
                                         start=True, stop=True)
                        as_ps = hps.tile([128, HEADS], F32, tag="as_ps")
                        nc.tensor.matmul(as_ps[:], lhsT, Wsrc_sb[:],
                                         start=True, stop=True)
                        stage = hpool.tile([128, EXT], F32, tag="hg_stage")
                        nc.vector.tensor_copy(stage[:, 0:HH], hg_ps[:])
                        nc.vector.tensor_copy(stage[:, HH:HH + HEADS], as_ps[:])
                        nc.vector.memset(stage[:, HH + HEADS:EXT], 0.0)
                        nc.sync.dma_start(
                            hg_ext[:].rearrange("(w p) f -> w p f", p=128)[blk * W + j],
                            stage[:])

            # ================= Phase D: GAT =================
            offA = offB = offT = 0
            if "D" in phases:
              with tc.tile_pool(name="gat_sb", bufs=2) as apool, \
                 tc.tile_pool(name="gat_sb1", bufs=1) as apool1, \
                 tc.tile_pool(name="gat_ps", bufs=2, space="PSUM") as aps, \
                 tc.tile_pool(name="gat_ps2", bufs=2, space="PSUM") as aps2:
                for j in range(W):
                    TAj, TBj = TA1[j], TB1[j]
                    T = TAj + TBj
                    pay = gather_window(apool, "gat_pay", e1_idxA_in, e1_idxB_in,
                                        hg_ext[:], hgB, offA, offB, TAj, TBj,
                                        EXT, Tmax1)
                    pay3 = pay[:].rearrange("p (t f) -> p t f", f=EXT)
                    n = T * 128
                    cit = apool.tile([128, Tmax1 * 8], I16, tag="gat_cidx")
                    nc.sync.dma_start(cit[:, :n // 16],
                                      e1_col_in[:, offT * 8: offT * 8 + n // 16])
                    adst_e = apool.tile([128, Tmax1 * 64], F32, tag="gat_adst")
                    adst3 = adst_e[:].rearrange("p (t f) -> p t f", f=64)
                    nc.gpsimd.dma_gather(adst3[:, 0:T], adst_tab[:],
                                         cit[:, :n // 16],
                                         num_idxs=n, num_idxs_reg=n, elem_size=64,
                                         single_packet=False)
                    G = build_G(apool1, "gat_G", e1_lcol_sb, None, offT, T)
                    G3 = G[:].rearrange("p (t c) -> p t c", c=128)
                    # e = lrelu(asrc + adst); ex = exp(e)
                    ex_sb = apool1.tile([128, Tmax1 * HEADS], F32, tag="gat_ex")
                    ex3 = ex_sb[:].rearrange("p (t h) -> p t h", h=HEADS)
                    nc.vector.tensor_tensor(ex3[:, 0:T],
                                            pay3[:, 0:T, HH:HH + HEADS],
                                            adst3[:, 0:T, 0:HEADS], AOT.add)
                    tmp_sb = apool1.tile([128, Tmax1 * HEADS], F32, tag="gat_tmp")
                    tmp3 = tmp_sb[:].rearrange("p (t h) -> p t h", h=HEADS)
                    nc.vector.tensor_scalar_mul(tmp3[:, 0:T], ex3[:, 0:T], 0.2)
                    nc.vector.tensor_tensor(ex3[:, 0:T], ex3[:, 0:T],
                                            tmp3[:, 0:T], AOT.max)
                    nc.scalar.activation(ex3[:, 0:T], ex3[:, 0:T], ACTF.Exp)
                    # Vw = hg * ex (broadcast 64 per head)
                    Vw = apool1.tile([128, Tmax1 * HH], F32, tag="gat_Vw")
                    vw4 = _split_last(
                        Vw[:].rearrange("p (t f) -> p t f", f=HH)[:, 0:T], HEADS, H)
                    hg4 = _split_last(pay3[:, 0:T, 0:HH], HEADS, H)
                    exb = _bc(ex3[:, 0:T], H)
                    nc.vector.tensor_tensor(vw4, hg4, exb, AOT.mult)
                    Vw3 = Vw[:].rearrange("p (t f) -> p t f", f=HH)
                    agg = aps.tile([128, HH], F32, tag="gat_agg")
                    agg_d = aps.tile([128, HEADS], F32, tag="gat_agg_d")
                    for t in range(T):
                        nc.tensor.matmul(agg[:], G3[:, t], Vw3[:, t],
                                         start=(t == 0), stop=(t == T - 1))
                        nc.tensor.matmul(agg_d[:], G3[:, t], ex3[:, t],
                                         start=(t == 0), stop=(t == T - 1))
                    den = apool.tile([128, HEADS], F32, tag="gat_den")
                    nc.vector.tensor_scalar_max(den[:], agg_d[:], 1e-30)
                    rden = apool.tile([128, HEADS], F32, tag="gat_rden")
                    nc.vector.reciprocal(rden[:], den[:])
                    acc = apool.tile([128, H], F32, tag="gat_acc")
                    nc.vector.tensor_scalar(acc[:], agg[:, 0:H], rden[:, 0:1],
                                            None, AOT.mult)
                    tmph = apool.tile([128, H], F32, tag="gat_tmph")
                    for hh in range(1, HEADS):
                        nc.vector.tensor_scalar(tmph[:], agg[:, hh * H:(hh + 1) * H],
                                                rden[:, hh:hh + 1], None, AOT.mult)
                        nc.vector.tensor_tensor(acc[:], acc[:], tmph[:], AOT.add)
                    h2_sb = apool.tile([128, H], F32, tag="gat_h2")
                    nc.vector.tensor_scalar_mul(h2_sb[:], acc[:], 1.0 / HEADS)
                    nc.vector.tensor_tensor(h2_sb[:], h2_sb[:], bgr_sb[:], AOT.add)
                    nc.vector.tensor_scalar_max(h2_sb[:], h2_sb[:], 0.0)
                    nc.sync.dma_start(
                        h2_shard[:].rearrange("(w p) f -> w p f", p=128)[j],
                        h2_sb[:])
                    tr_ps = aps2.tile([H, 128], F32, tag="gat_tr")
                    nc.tensor.transpose(tr_ps[:], h2_sb[:], ident[:])
                    nc.vector.tensor_copy(h2T_own[:, j * 128:(j + 1) * 128], tr_ps[:])
                    offA += TAj
                    offB += TBj
                    offT += T

            if "2" in phases:
                nc.gpsimd.collective_compute(
                    "AllGather", AOT.bypass,
                    replica_groups=[list(range(n_cores))],
                    ins=[h2_shard.opt()], outs=[h2_full.opt()])

            # ================= Phase E: SAGE + heads =================
            offA = offB = offT = 0
            if "E" in phases:
              with tc.tile_pool(name="sg_sb", bufs=2) as spool, \
                 tc.tile_pool(name="sg_ps", bufs=2, space="PSUM") as sps, \
                 tc.tile_pool(name="sg_ps2", bufs=1, space="PSUM") as sps2:
                for j in range(W):
                    TAj, TBj = TA0[j], TB0[j]
                    T = TAj + TBj
                    maggT_sb = spool.tile([H, 128], F32, tag="sg_maggT_sb")
                    if T > 0:
                        pay = gather_window(spool, "sg_pay", e0_idxA_in,
                                            e0_idxB_in, h2_full[:], h2B,
                                            offA, offB, TAj, TBj, H, Tmax0)
                        pay3 = pay[:].rearrange("p (t f) -> p t f", f=H)
                        G = build_G(spool, "sg_G", e0_lcol_sb, None, offT, T)
                        G3 = G[:].rearrange("p (t c) -> p t c", c=128)
                        maggT = sps.tile([H, 128], F32, tag="sg_maggT")
                        for t in range(T):
                            nc.tensor.matmul(maggT[:], pay3[:, t], G3[:, t],
                                             start=(t == 0), stop=(t == T - 1))
                        nc.vector.tensor_copy(maggT_sb[:], maggT[:])
                    else:
                        nc.vector.memset(maggT_sb[:], 0.0)
                    ml_ps = sps2.tile([128, H], F32, tag="sg_ml")
                    nc.tensor.matmul(ml_ps[:], maggT_sb[:], Wl_sb[:],
                                     start=True, stop=True)
                    wr_ps = sps2.tile([128, H], F32, tag="sg_wr")
                    nc.tensor.matmul(wr_ps[:], h2T_own[:, j * 128:(j + 1) * 128],
                                     Wr_sb[:], start=True, stop=True)
                    emb_sb = spool.tile([128, H], F32, tag="sg_emb")
                    nc.vector.tensor_scalar(emb_sb[:], ml_ps[:],
                                            inv0o_sb[:, j:j + 1], None, AOT.mult)
                    nc.vector.tensor_tensor(emb_sb[:], emb_sb[:], wr_ps[:], AOT.add)
                    nc.vector.tensor_tensor(emb_sb[:], emb_sb[:], blr_sb[:], AOT.add)
                    nc.sync.dma_start(
                        emb_out[:].rearrange("(w p) f -> w p f", p=128)[j],
                        emb_sb[:])
                    embT_ps = sps2.tile([H, 128], F32, tag="sg_embT")
                    nc.tensor.transpose(embT_ps[:], emb_sb[:], ident[:])
                    embT_sb = spool.tile([H, 128], F32, tag="sg_embT_sb")
                    nc.vector.tensor_copy(embT_sb[:], embT_ps[:])
                    hid_ps = sps2.tile([128, 64], F32, tag="sg_hid")
                    nc.tensor.matmul(hid_ps[:], embT_sb[:], ar1w_sb[:],
                                     start=True, stop=True)
                    hid_sb = spool.tile([128, 64], F32, tag="sg_hid_sb")
                    nc.vector.tensor_tensor(hid_sb[:], hid_ps[:], ar1br_sb[:],
                                            AOT.add)
                    nc.vector.tensor_scalar_max(hid_sb[:], hid_sb[:], 0.0)
                    hidT_ps = sps2.tile([64, 128], F32, tag="sg_hidT")
                    nc.tensor.transpose(hidT_ps[:], hid_sb[:], ident[:])
                    hidT_sb = spool.tile([64, 128], F32, tag="sg_hidT_sb")
                    nc.vector.tensor_copy(hidT_sb[:], hidT_ps[:])
                    o_ps = sps2.tile([128, 2], F32, tag="sg_o")
                    nc.tensor.matmul(o_ps[:], hidT_sb[:], ar2w_sb[:],
                                     start=True, stop=True)
                    o_sb = spool.tile([128, 2], F32, tag="sg_o_sb")
                    nc.vector.tensor_tensor(o_sb[:], o_ps[:], ar2br_sb[:], AOT.add)
                    nc.scalar.activation(o_sb[:], o_sb[:], ACTF.Sigmoid)
                    nc.sync.dma_start(
                        outp_out[:].rearrange("(w p) f -> w p f", p=128)[j],
                        o_sb[:])
                    offA += TAj
                    offB += TBj
                    offT += T

            if "E" not in phases:
                with tc.tile_pool(name="dummy_sb", bufs=1) as dpool:
                    z64 = dpool.tile([128, H], F32, tag="z64")
                    nc.vector.memset(z64[:], 0.0)
                    z2 = dpool.tile([128, 2], F32, tag="z2")
                    nc.vector.memset(z2[:], 0.0)
                    for j in range(W):
                        nc.sync.dma_start(
                            emb_out[:].rearrange("(w p) f -> w p f", p=128)[j], z64[:])
                        nc.sync.dma_start(
                            outp_out[:].rearrange("(w p) f -> w p f", p=128)[j], z2[:])

    if not nc.is_finalized():
        nc.finalize()
    return nc


# ----------------------------------------------------------------------------
# Top-level kernel
# ----------------------------------------------------------------------------

def _prep_inputs(x, edge_index, W1, b1, Wg, bg, att_src, att_dst, Wl, bl, Wr,
                 a1_w, a1_b, a2_w, a2_b, r1_w, r1_b, r2_w, r2_b,
                 n_cores=NCORES):
    x = np.ascontiguousarray(np.asarray(x, np.float32))
    N, F_IN = x.shape
    H = np.asarray(W1).shape[1]
    HEADS = np.asarray(att_src).shape[0]
    npc = -(-N // (n_cores * P)) * P
    W = npc // P
    N_pad = npc * n_cores

    row0 = np.asarray(edge_index[0]).astype(np.int64)
    col0 = np.asarray(edge_index[1]).astype(np.int64)
    loops = np.arange(N, dtype=np.int64)
    colL = np.concatenate([col0, loops])
    rowL = np.concatenate([row0, loops])
    deg = np.bincount(colL, minlength=N_pad).astype(np.float64)
    dinv = np.where(deg > 0, deg ** -0.5, 0.0).astype(np.float32)
    deg0 = np.bincount(col0, minlength=N_pad).astype(np.float64)
    inv0 = (1.0 / np.maximum(deg0, 1.0)).astype(np.float32)

    e1 = _build_edge_set(rowL, colL, dinv[rowL], W, npc, N)
    e0 = _build_edge_set(row0, col0, None, W, npc, N)

    Wg_ = np.ascontiguousarray(np.asarray(Wg, np.float32))
    att_s = np.asarray(att_src, np.float32)
    att_d = np.asarray(att_dst, np.float32)
    Wg3 = Wg_.reshape(H, HEADS, H)
    Wsrc = np.ascontiguousarray(np.einsum("khc,hc->kh", Wg3, att_s).astype(np.float32))
    Wdst = np.ascontiguousarray(np.einsum("khc,hc->kh", Wg3, att_d).astype(np.float32))
    ar1w = np.ascontiguousarray(np.concatenate(
        [np.asarray(a1_w, np.float32), np.asarray(r1_w, np.float32)], axis=1))
    ar2w = np.zeros((64, 2), np.float32)
    ar2w[0:32, 0] = np.asarray(a2_w, np.float32)[:, 0]
    ar2w[32:64, 1] = np.asarray(r2_w, np.float32)[:, 0]
    rep = lambda v: np.ascontiguousarray(
        np.tile(np.asarray(v, np.float32)[None, :], (128, 1)))
    ar1b = np.concatenate([np.asarray(a1_b, np.float32),
                           np.asarray(r1_b, np.float32)])
    ar2b = np.array([float(np.asarray(a2_b).reshape(-1)[0]),
                     float(np.asarray(r2_b).reshape(-1)[0])], np.float32)

    shared = dict(
        x=x,
        W1=np.ascontiguousarray(np.asarray(W1, np.float32)),
        Wg=Wg_, Wsrc=Wsrc, Wdst=Wdst,
        Wl=np.ascontiguousarray(np.asarray(Wl, np.float32)),
        Wr=np.ascontiguousarray(np.asarray(Wr, np.float32)),
        ar1w=ar1w, ar2w=ar2w,
        b1r=rep(b1), bgr=rep(bg), blr=rep(bl),
        ar1br=rep(ar1b), ar2br=rep(ar2b),
    )
    in_maps = []
    for c in range(n_cores):
        m = dict(shared)
        m["dinv_own"] = np.ascontiguousarray(
            dinv[c * npc:(c + 1) * npc].reshape(W, P).T)
        m["inv0_own"] = np.ascontiguousarray(
            inv0[c * npc:(c + 1) * npc].reshape(W, P).T)
        pc1, pc0 = e1["per_core"][c], e0["per_core"][c]
        m["e1_idxA"] = pc1["idxA"]
        m["e1_idxB"] = pc1["idxB"]
        m["e1_col"] = pc1["colidx"]
        m["e1_lcol"] = pc1["lcol"]
        m["e1_dinw"] = pc1["dinw"]
        m["e0_idxA"] = pc0["idxA"]
        m["e0_idxB"] = pc0["idxB"]
        m["e0_lcol"] = pc0["lcol"] if pc0["lcol"].shape[1] else np.zeros((128, 1), np.float32)
        in_maps.append(m)

    dims = dict(N=N, F_IN=F_IN, H=H, HEADS=HEADS, W=W, npc=npc)
    return dims, e1, e0, in_maps


_program_cache = {}


def kernel(x, edge_index, W1, b1, Wg, bg, att_src, att_dst, Wl, bl, Wr,
           a1_w, a1_b, a2_w, a2_b, r1_w, r1_b, r2_w, r2_b):
    x = np.asarray(x)
    N = x.shape[0]
    dims, e1, e0, in_maps = _prep_inputs(
        x, edge_index, W1, b1, Wg, bg, att_src, att_dst, Wl, bl, Wr,
        a1_w, a1_b, a2_w, a2_b, r1_w, r1_b, r2_w, r2_b)
    key = (dims["N"], dims["F_IN"], dims["H"], dims["HEADS"],
           tuple(e1["TA"]), tuple(e1["TB"]), tuple(e0["TA"]), tuple(e0["TB"]))
    if key not in _program_cache:
        _program_cache[key] = build_program(
            dims["N"], dims["F_IN"], dims["H"], dims["HEADS"],
            dims["W"], dims["npc"], e1, e0)
    nc = _program_cache[key]
    res = run_bass_kernel_spmd(nc, in_maps, list(range(NCORES)))
    emb = np.concatenate([res.results[c]["emb_own"] for c in range(NCORES)],
                         axis=0)[:N]
    outp = np.concatenate([res.results[c]["outp_own"] for c in range(NCORES)],
                          axis=0)[:N]
    return emb, np.ascontiguousarray(outp[:, 0:1]), np.ascontiguousarray(outp[:, 1:2])


# revision 8
# speedup vs baseline: 1.1280x; 1.1280x over previous
"""Trainium2 Bass kernel for CloudGNN (GCN -> GAT -> SAGE -> heads).

Strategy (graph/data parallel over 8 NeuronCores):
  - Nodes padded to N_pad = 8 * nodes_per_core, partitioned contiguously;
    each core owns W windows of 128 target nodes.
  - Edges assigned by target (col) window. Per (core, window) edges split by
    source row < 32768 (dma_gather idx is int16) into A/B groups, each padded
    to a multiple of 128 (an "edge tile"). Tile counts per window slot are
    global maxima over cores so one SPMD program fits every core.
  - Per edge tile [128 edges]: source payload rows fetched with dma_gather;
    a one-hot G[e, c] = (lcol[e] == c) built on-device (iota + is_equal);
    segment-sum = PE matmul accumulated in PSUM across the window's tiles.
  - GCN aggregates raw x (linearity: (sum w_e x_row) @ W1); dinv[row] folded
    into G, dinv[col] applied on output. Self-loops are ordinary edges.
  - GAT gathers hg_ext = [hg (256) | a_src (4) | pad] rows; a_dst[col] comes
    from a col-indexed core-local gather; softmax computed unnormalized
    (exp without max-subtraction; numerator + denominator are segment sums
    in the same PSUM matmul; divide densely at the end).
  - SAGE aggregates h2 rows over the no-self-loop edge set; 1/deg0 and
    Wl/Wr/bl applied densely per window.
  - Two AllGathers: h1^T blocks (for dense hg recompute) and h2 (SAGE table).
"""

import sys

sys.path.insert(0, "/opt/trn_rl_repo")

import numpy as np

import concourse.bass as bass
import concourse.bacc as bacc
import concourse.tile as tile
from concourse import mybir
from concourse.bass_utils import run_bass_kernel_spmd

F32 = mybir.dt.float32
BF16 = mybir.dt.bfloat16
I32 = mybir.dt.int32
I16 = mybir.dt.int16
AOT = mybir.AluOpType
ACTF = mybir.ActivationFunctionType

P = 128
NCORES = 8
TH = 32768  # int16 gather index limit
GAT_BF16 = False  # bf16 GAT payload/G: ~12% faster, rel err ~1.3e-3 vs ~1e-6


def _bc(ap, count):
    """Append a [0, count] broadcast dim to an AP."""
    return bass.AP(ap.tensor, ap.offset, list(ap.ap) + [[0, count]])


def _split_last(ap, h, f):
    """Replace last dim [1, h*f] of an AP with [f, h], [1, f]."""
    pat = list(ap.ap)
    assert pat[-1][0] == 1 and pat[-1][1] == h * f
    return bass.AP(ap.tensor, ap.offset, pat[:-1] + [[f, h], [1, f]])


# ----------------------------------------------------------------------------
# Host preprocessing
# ----------------------------------------------------------------------------

def _wrap_idx(idx_flat):
    """int16 gather idx layout: idx i at [i%16, i//16], replicated x8 -> 128 rows."""
    n = idx_flat.shape[0]
    assert n % 128 == 0
    w = idx_flat.reshape(n // 16, 16).T.astype(np.int16)
    return np.tile(w, (8, 1))


def _build_edge_set(row, col, weight, W, npc, table_rows):
    row = row.astype(np.int64)
    col = col.astype(np.int64)
    w_abs = col // P
    core = w_abs // W
    slot = w_abs % W
    half = (row >= TH).astype(np.int64)
    key = (core * W + slot) * 2 + half
    order = np.argsort(key, kind="stable")
    row_s, col_s = row[order], col[order]
    wt_s = weight[order] if weight is not None else None

    counts = np.bincount(key, minlength=NCORES * W * 2).reshape(NCORES, W, 2)
    tiles = -(-counts // P)
    TA = tiles[:, :, 0].max(axis=0)
    TB = tiles[:, :, 1].max(axis=0)
    sumTA, sumTB = int(TA.sum()), int(TB.sum())
    sumT = sumTA + sumTB

    starts = np.zeros(NCORES * W * 2 + 1, dtype=np.int64)
    np.cumsum(counts.reshape(-1), out=starts[1:])

    per_core = []
    for c in range(NCORES):
        idxA = np.zeros(max(1, sumTA) * P, dtype=np.int64)
        idxB = np.zeros(max(1, sumTB) * P, dtype=np.int64)
        lcol = np.full(sumT * P, -1.0, dtype=np.float32)
        dinw = np.zeros(sumT * P, dtype=np.float32)
        lcolc = np.zeros(sumT * P, dtype=np.int64)
        offA = offB = offT = 0
        for j in range(W):
            for h in (0, 1):
                Tn = int(TA[j] if h == 0 else TB[j])
                k = (c * W + j) * 2 + h
                s, e = starts[k], starts[k + 1]
                n = int(e - s)
                if h == 0:
                    idxA[offA:offA + n] = row_s[s:e]
                    offA += Tn * P
                else:
                    idxB[offB:offB + n] = row_s[s:e] - TH
                    offB += Tn * P
                lcol[offT:offT + n] = (col_s[s:e] % P).astype(np.float32)
                lcolc[offT:offT + n] = col_s[s:e] - c * npc
                if wt_s is not None:
                    dinw[offT:offT + n] = wt_s[s:e]
                else:
                    dinw[offT:offT + n] = 1.0
                offT += Tn * P
        assert idxA.max(initial=0) < table_rows
        assert idxB.max(initial=0) < max(1, table_rows - TH)
        assert lcolc.max(initial=0) < npc
        per_core.append(dict(
            idxA=_wrap_idx(idxA),
            idxB=_wrap_idx(idxB),
            colidx=_wrap_idx(lcolc) if sumT else np.zeros((128, 1), np.int16),
            lcol=np.ascontiguousarray(lcol.reshape(sumT, P).T),
            dinw=np.ascontiguousarray(dinw.reshape(sumT, P).T),
        ))
    return dict(TA=TA.tolist(), TB=TB.tolist(), per_core=per_core,
                sumTA=sumTA, sumTB=sumTB, sumT=sumT)


# ----------------------------------------------------------------------------
# Device program
# ----------------------------------------------------------------------------

def build_program(N, F_IN, H, HEADS, W, npc, e1, e0, n_cores=NCORES,
                  phases="B1CD2E"):
    N_pad = npc * n_cores
    HH = HEADS * H                       # 256
    GDT = BF16 if GAT_BF16 else F32      # GAT payload/G dtype
    # hg_ext row: hg | a_src (4 x f32) | pad.  In bf16 mode a_src is packed
    # as raw f32 bytes occupying 8 bf16 slots (read back via bitcast).
    EXT = (HH + 128) if GAT_BF16 else (HH + 64)
    ASRC_OFF = HH // 2 if GAT_BF16 else HH  # a_src offset in f32 units
    TA1, TB1 = e1["TA"], e1["TB"]
    TA0, TB0 = e0["TA"], e0["TB"]
    sT1, sT0 = e1["sumT"], e0["sumT"]
    Tmax1 = max(TA1[j] + TB1[j] for j in range(W))
    Tmax0 = max((TA0[j] + TB0[j] for j in range(W)), default=1)
    TmaxH = max(max(TA1), max(TB1), 1)  # max tiles in any single gather

    nc = bacc.Bacc("TRN2", target_bir_lowering=False, debug=False,
                   num_devices=n_cores)

    x_in = nc.dram_tensor("x", [N, F_IN], F32, kind="ExternalInput")
    W1_in = nc.dram_tensor("W1", [F_IN, H], F32, kind="ExternalInput")
    Wg_in = nc.dram_tensor("Wg", [H, HH], F32, kind="ExternalInput")
    Wsrc_in = nc.dram_tensor("Wsrc", [H, HEADS], F32, kind="ExternalInput")
    Wdst_in = nc.dram_tensor("Wdst", [H, HEADS], F32, kind="ExternalInput")
    Wl_in = nc.dram_tensor("Wl", [H, H], F32, kind="ExternalInput")
    Wr_in = nc.dram_tensor("Wr", [H, H], F32, kind="ExternalInput")
    ar1w_in = nc.dram_tensor("ar1w", [H, 64], F32, kind="ExternalInput")
    ar2w_in = nc.dram_tensor("ar2w", [64, 2], F32, kind="ExternalInput")
    b1r_in = nc.dram_tensor("b1r", [128, H], F32, kind="ExternalInput")
    bgr_in = nc.dram_tensor("bgr", [128, H], F32, kind="ExternalInput")
    blr_in = nc.dram_tensor("blr", [128, H], F32, kind="ExternalInput")
    ar1br_in = nc.dram_tensor("ar1br", [128, 64], F32, kind="ExternalInput")
    ar2br_in = nc.dram_tensor("ar2br", [128, 2], F32, kind="ExternalInput")
    dinvo_in = nc.dram_tensor("dinv_own", [128, W], F32, kind="ExternalInput")
    inv0o_in = nc.dram_tensor("inv0_own", [128, W], F32, kind="ExternalInput")
    e1_idxA_in = nc.dram_tensor("e1_idxA", [128, max(1, e1["sumTA"]) * 8], I16, kind="ExternalInput")
    e1_idxB_in = nc.dram_tensor("e1_idxB", [128, max(1, e1["sumTB"]) * 8], I16, kind="ExternalInput")
    e1_col_in = nc.dram_tensor("e1_col", [128, max(1, sT1) * 8], I16, kind="ExternalInput")
    e1_lcol_in = nc.dram_tensor("e1_lcol", [128, sT1], F32, kind="ExternalInput")
    e1_dinw_in = nc.dram_tensor("e1_dinw", [128, sT1], F32, kind="ExternalInput")
    e0_idxA_in = nc.dram_tensor("e0_idxA", [128, max(1, e0["sumTA"]) * 8], I16, kind="ExternalInput")
    e0_idxB_in = nc.dram_tensor("e0_idxB", [128, max(1, e0["sumTB"]) * 8], I16, kind="ExternalInput")
    e0_lcol_in = nc.dram_tensor("e0_lcol", [128, max(1, sT0)], F32, kind="ExternalInput")

    emb_out = nc.dram_tensor("emb_own", [npc, H], F32, kind="ExternalOutput")
    outp_out = nc.dram_tensor("outp_own", [npc, 2], F32, kind="ExternalOutput")

    with tile.TileContext(nc) as tc:
        with tc.tile_pool(name="dram", bufs=1, space="DRAM") as dram, \
             tc.tile_pool(name="const", bufs=1) as cpool, \
             tc.tile_pool(name="persist", bufs=1) as ppool:

            hg_ext = dram.tile([N_pad, EXT], GDT)
            adst_tab = dram.tile([npc, 64], F32)
            h1T_shard = dram.tile([H, npc], F32)
            h1T_blocks = dram.tile([H * n_cores, npc], F32)
            h2_shard = dram.tile([npc, H], F32)
            h2_full = dram.tile([N_pad, H], F32)

            # ---- constants ----
            iota_i = cpool.tile([128, Tmax1 * 128], I32, tag="iota_i")
            nc.gpsimd.iota(iota_i[:], pattern=[[0, Tmax1], [1, 128]], base=0,
                           channel_multiplier=0)
            iota_rep = cpool.tile([128, Tmax1 * 128], F32, tag="iota_rep")
            nc.vector.tensor_copy(iota_rep[:], iota_i[:])
            pidx_i = cpool.tile([128, 1], I32, tag="pidx_i")
            nc.gpsimd.iota(pidx_i[:], pattern=[[0, 1]], base=0, channel_multiplier=1)
            pidx_f = cpool.tile([128, 1], F32, tag="pidx_f")
            nc.vector.tensor_copy(pidx_f[:], pidx_i[:])
            ident = cpool.tile([128, 128], F32, tag="ident")
            nc.vector.tensor_scalar(ident[:], iota_rep[:, 0:128], pidx_f[:], None,
                                    AOT.is_equal)

            def load_const(name, src, shape):
                t = cpool.tile(list(shape), F32, tag=name)
                nc.sync.dma_start(t[:], src[:])
                return t

            W1_sb = load_const("W1c", W1_in, (F_IN, H))
            Wg_sb = load_const("Wgc", Wg_in, (H, HH))
            Wsrc_sb = load_const("Wsrcc", Wsrc_in, (H, HEADS))
            Wdst_sb = load_const("Wdstc", Wdst_in, (H, HEADS))
            Wl_sb = load_const("Wlc", Wl_in, (H, H))
            Wr_sb = load_const("Wrc", Wr_in, (H, H))
            ar1w_sb = load_const("ar1wc", ar1w_in, (H, 64))
            ar2w_sb = load_const("ar2wc", ar2w_in, (64, 2))
            b1r_sb = load_const("b1rc", b1r_in, (128, H))
            bgr_sb = load_const("bgrc", bgr_in, (128, H))
            blr_sb = load_const("blrc", blr_in, (128, H))
            ar1br_sb = load_const("ar1brc", ar1br_in, (128, 64))
            ar2br_sb = load_const("ar2brc", ar2br_in, (128, 2))
            dinvo_sb = load_const("dinvoc", dinvo_in, (128, W))
            inv0o_sb = load_const("inv0oc", inv0o_in, (128, W))
            e1_lcol_sb = load_const("e1lcolc", e1_lcol_in, (128, sT1))
            e1_dinw_sb = load_const("e1dinwc", e1_dinw_in, (128, sT1))
            e0_lcol_sb = load_const("e0lcolc", e0_lcol_in, (128, max(1, sT0)))

            h1T_own = ppool.tile([H, npc], F32, tag="h1T_own")
            h2T_own = ppool.tile([H, npc], F32, tag="h2T_own")

            def build_G(pool, bufs_tag, lcol_sb, wt_sb, t_off, T, dt=F32):
                G = pool.tile([128, Tmax1 * 128], dt, tag=bufs_tag)
                Gv = G[:].rearrange("p (t c) -> p t c", c=128)[:, 0:T]
                i0 = iota_rep[:].rearrange("p (t c) -> p t c", c=128)[:, 0:T]
                l1 = _bc(lcol_sb[:, t_off:t_off + T], 128)
                nc.vector.tensor_tensor(Gv, i0, l1, AOT.is_equal)
                if wt_sb is not None:
                    w1 = _bc(wt_sb[:, t_off:t_off + T], 128)
                    nc.vector.tensor_tensor(Gv, Gv, w1, AOT.mult)
                return G

            def gather_window(pool, tag, idxA_in, idxB_in, tabA, tabB,
                              offA, offB, TAj, TBj, elem, tmax, dt=F32):
                """A+B gathers for one window -> payload tile [128, tmax*elem]."""
                pay = pool.tile([128, tmax * elem], dt, tag=tag)
                pay3 = pay[:].rearrange("p (t f) -> p t f", f=elem)
                for h, (Tn, idx_in, off_t, tab) in enumerate((
                    (TAj, idxA_in, offA, tabA),
                    (TBj, idxB_in, offB, tabB),
                )):
                    if Tn == 0:
                        continue
                    n = Tn * 128
                    it = pool.tile([128, TmaxH * 8], I16, tag=tag + "_idx")
                    nc.sync.dma_start(it[:, :n // 16],
                                      idx_in[:, off_t * 8: off_t * 8 + n // 16])
                    dst = pay3[:, 0:Tn] if h == 0 else pay3[:, TAj:TAj + Tn]
                    nc.gpsimd.dma_gather(dst, tab, it[:, :n // 16],
                                         num_idxs=n, num_idxs_reg=n,
                                         elem_size=elem, single_packet=False)
                return pay

            xB = x_in[TH:N, :] if N > TH else None
            hgB = hg_ext[:][TH:N_pad, :] if N_pad > TH else None
            h2B = h2_full[:][TH:N_pad, :] if N_pad > TH else None

            # ================= Phase B: GCN =================
            offA = offB = offT = 0
            if "B" in phases:
              with tc.tile_pool(name="gcn_sb", bufs=2) as gpool, \
                 tc.tile_pool(name="gcn_ps", bufs=2, space="PSUM") as gps, \
                 tc.tile_pool(name="gcn_ps2", bufs=1, space="PSUM") as gps2:
                for j in range(W):
                    TAj, TBj = TA1[j], TB1[j]
                    T = TAj + TBj
                    pay = gather_window(gpool, "gcn_pay", e1_idxA_in, e1_idxB_in,
                                        x_in[:], xB, offA, offB, TAj, TBj,
                                        F_IN, Tmax1)
                    pay3 = pay[:].rearrange("p (t f) -> p t f", f=F_IN)
                    G = build_G(gpool, "gcn_G", e1_lcol_sb, e1_dinw_sb, offT, T)
                    G3 = G[:].rearrange("p (t c) -> p t c", c=128)
                    aggT = gps.tile([128, 128], F32, tag="gcn_aggT")
                    for t in range(T):
                        nc.tensor.matmul(aggT[:], pay3[:, t], G3[:, t],
                                         start=(t == 0), stop=(t == T - 1))
                    aggT_sb = gpool.tile([128, 128], F32, tag="gcn_aggT_sb")
                    nc.vector.tensor_copy(aggT_sb[:], aggT[:])
                    h1_ps = gps2.tile([128, H], F32, tag="gcn_h1")
                    nc.tensor.matmul(h1_ps[:], aggT_sb[:], W1_sb[:],
                                     start=True, stop=True)
                    h1_sb = gpool.tile([128, H], F32, tag="gcn_h1_sb")
                    nc.vector.tensor_scalar_mul(h1_sb[:], h1_ps[:],
                                                dinvo_sb[:, j:j + 1])
                    nc.vector.tensor_tensor(h1_sb[:], h1_sb[:], b1r_sb[:], AOT.add)
                    nc.vector.tensor_scalar_max(h1_sb[:], h1_sb[:], 0.0)
                    tr_ps = gps2.tile([H, 128], F32, tag="gcn_tr")
                    nc.tensor.transpose(tr_ps[:], h1_sb[:], ident[:])
                    nc.vector.tensor_copy(h1T_own[:, j * 128:(j + 1) * 128], tr_ps[:])
                    adst_ps = gps2.tile([128, HEADS], F32, tag="gcn_adst")
                    nc.tensor.matmul(adst_ps[:], h1T_own[:, j * 128:(j + 1) * 128],
                                     Wdst_sb[:], start=True, stop=True)
                    adst_sb = gpool.tile([128, 64], F32, tag="gcn_adst_sb")
                    nc.vector.memset(adst_sb[:], 0.0)
                    nc.vector.tensor_copy(adst_sb[:, 0:HEADS], adst_ps[:])
                    nc.sync.dma_start(
                        adst_tab[:].rearrange("(w p) f -> w p f", p=128)[j],
                        adst_sb[:])
                    offA += TAj
                    offB += TBj
                    offT += T
                nc.sync.dma_start(h1T_shard[:], h1T_own[:])

            if "1" in phases:
                nc.gpsimd.collective_compute(
                    "AllGather", AOT.bypass,
                    replica_groups=[list(range(n_cores))],
                    ins=[h1T_shard.opt()], outs=[h1T_blocks.opt()])

            # ================= Phase C: hg_ext table =================
            if "C" in phases:
              with tc.tile_pool(name="hg_sb", bufs=2) as hpool, \
                 tc.tile_pool(name="hg_ps", bufs=2, space="PSUM") as hps:
                for blk in range(n_cores):
                    h1T_blk = hpool.tile([H, npc], F32, tag="h1T_blk")
                    nc.sync.dma_start(h1T_blk[:],
                                      h1T_blocks[:][blk * H:(blk + 1) * H, :])
                    for j in range(W):
                        lhsT = h1T_blk[:, j * 128:(j + 1) * 128]
                        hg_ps = hps.tile([128, HH], F32, tag="hg_ps")
                        nc.tensor.matmul(hg_ps[:], lhsT, Wg_sb[:],
                                         start=True, stop=True)
                        as_ps = hps.tile([128, HEADS], F32, tag="as_ps")
                        nc.tensor.matmul(as_ps[:], lhsT, Wsrc_sb[:],
                                         start=True, stop=True)
                        stage = hpool.tile([128, EXT], GDT, tag="hg_stage")
                        nc.vector.tensor_copy(stage[:, 0:HH], hg_ps[:])
                        stage_f = stage[:].bitcast(F32) if GAT_BF16 else stage[:]
                        nc.vector.tensor_copy(
                            stage_f[:, ASRC_OFF:ASRC_OFF + HEADS], as_ps[:])
                        pad0 = HH + (2 * HEADS if GAT_BF16 else HEADS)
                        nc.vector.memset(stage[:, pad0:EXT], 0.0)
                        nc.sync.dma_start(
                            hg_ext[:].rearrange("(w p) f -> w p f", p=128)[blk * W + j],
                            stage[:])

            # ================= Phase D: GAT =================
            offA = offB = offT = 0
            if "D" in phases:
              with tc.tile_pool(name="gat_sb", bufs=2) as apool, \
                 tc.tile_pool(name="gat_sb1", bufs=1) as apool1, \
                 tc.tile_pool(name="gat_ps", bufs=2, space="PSUM") as aps, \
                 tc.tile_pool(name="gat_ps2", bufs=2, space="PSUM") as aps2:
                for j in range(W):
                    TAj, TBj = TA1[j], TB1[j]
                    T = TAj + TBj
                    pay = gather_window(apool, "gat_pay", e1_idxA_in, e1_idxB_in,
                                        hg_ext[:], hgB, offA, offB, TAj, TBj,
                                        EXT, Tmax1, dt=GDT)
                    pay3 = pay[:].rearrange("p (t f) -> p t f", f=EXT)
                    payf = pay[:].bitcast(F32) if GAT_BF16 else pay[:]
                    payf3 = payf.rearrange(
                        "p (t f) -> p t f", f=EXT // 2 if GAT_BF16 else EXT)
                    n = T * 128
                    cit = apool.tile([128, Tmax1 * 8], I16, tag="gat_cidx")
                    nc.sync.dma_start(cit[:, :n // 16],
                                      e1_col_in[:, offT * 8: offT * 8 + n // 16])
                    adst_e = apool.tile([128, Tmax1 * 64], F32, tag="gat_adst")
                    adst3 = adst_e[:].rearrange("p (t f) -> p t f", f=64)
                    nc.gpsimd.dma_gather(adst3[:, 0:T], adst_tab[:],
                                         cit[:, :n // 16],
                                         num_idxs=n, num_idxs_reg=n, elem_size=64,
                                         single_packet=False)
                    G = build_G(apool1, "gat_G", e1_lcol_sb, None, offT, T,
                                dt=GDT)
                    G3 = G[:].rearrange("p (t c) -> p t c", c=128)
                    # e = lrelu(asrc + adst) in f32; ex = exp(e) -> bf16
                    e_sb = apool1.tile([128, Tmax1 * HEADS], F32, tag="gat_e")
                    e3 = e_sb[:].rearrange("p (t h) -> p t h", h=HEADS)
                    nc.vector.tensor_tensor(e3[:, 0:T],
                                            payf3[:, 0:T, ASRC_OFF:ASRC_OFF + HEADS],
                                            adst3[:, 0:T, 0:HEADS], AOT.add)
                    tmp_sb = apool1.tile([128, Tmax1 * HEADS], F32, tag="gat_tmp")
                    tmp3 = tmp_sb[:].rearrange("p (t h) -> p t h", h=HEADS)
                    nc.vector.tensor_scalar_mul(tmp3[:, 0:T], e3[:, 0:T], 0.2)
                    nc.vector.tensor_tensor(e3[:, 0:T], e3[:, 0:T],
                                            tmp3[:, 0:T], AOT.max)
                    # merged rhs buffer [128, T, 260] bf16: Vw | ex
                    HX = HH + HEADS
                    vwex = apool1.tile([128, Tmax1 * HX], GDT, tag="gat_vwex")
                    vx3 = vwex[:].rearrange("p (t f) -> p t f", f=HX)
                    ex3 = vx3[:, 0:Tmax1, HH:HX]
                    nc.scalar.activation(ex3[:, 0:T] if False else vx3[:, 0:T, HH:HX],
                                         e3[:, 0:T], ACTF.Exp)
                    vw4 = _split_last(vx3[:, 0:T, 0:HH], HEADS, H)
                    hg4 = _split_last(pay3[:, 0:T, 0:HH], HEADS, H)
                    exb = _bc(vx3[:, 0:T, HH:HX], H)
                    nc.vector.tensor_tensor(vw4, hg4, exb, AOT.mult)
                    agg = aps.tile([128, HX], F32, tag="gat_agg")
                    for t in range(T):
                        nc.tensor.matmul(agg[:], G3[:, t], vx3[:, t],
                                         start=(t == 0), stop=(t == T - 1))
                    den = apool.tile([128, HEADS], F32, tag="gat_den")
                    nc.vector.tensor_scalar_max(den[:], agg[:, HH:HX], 1e-30)
                    rden = apool.tile([128, HEADS], F32, tag="gat_rden")
                    nc.vector.reciprocal(rden[:], den[:])
                    acc = apool.tile([128, H], F32, tag="gat_acc")
                    nc.vector.tensor_scalar(acc[:], agg[:, 0:H], rden[:, 0:1],
                                            None, AOT.mult)
                    tmph = apool.tile([128, H], F32, tag="gat_tmph")
                    for hh in range(1, HEADS):
                        nc.vector.tensor_scalar(tmph[:], agg[:, hh * H:(hh + 1) * H],
                                                rden[:, hh:hh + 1], None, AOT.mult)
                        nc.vector.tensor_tensor(acc[:], acc[:], tmph[:], AOT.add)
                    h2_sb = apool.tile([128, H], F32, tag="gat_h2")
                    nc.vector.tensor_scalar_mul(h2_sb[:], acc[:], 1.0 / HEADS)
                    nc.vector.tensor_tensor(h2_sb[:], h2_sb[:], bgr_sb[:], AOT.add)
                    nc.vector.tensor_scalar_max(h2_sb[:], h2_sb[:], 0.0)
                    nc.sync.dma_start(
                        h2_shard[:].rearrange("(w p) f -> w p f", p=128)[j],
                        h2_sb[:])
                    tr_ps = aps2.tile([H, 128], F32, tag="gat_tr")
                    nc.tensor.transpose(tr_ps[:], h2_sb[:], ident[:])
                    nc.vector.tensor_copy(h2T_own[:, j * 128:(j + 1) * 128], tr_ps[:])
                    offA += TAj
                    offB += TBj
                    offT += T

            if "2" in phases:
                nc.gpsimd.collective_compute(
                    "AllGather", AOT.bypass,
                    replica_groups=[list(range(n_cores))],
                    ins=[h2_shard.opt()], outs=[h2_full.opt()])

            # ================= Phase E: SAGE + heads =================
            offA = offB = offT = 0
            if "E" in phases:
              with tc.tile_pool(name="sg_sb", bufs=2) as spool, \
                 tc.tile_pool(name="sg_ps", bufs=2, space="PSUM") as sps, \
                 tc.tile_pool(name="sg_ps2", bufs=1, space="PSUM") as sps2:
                for j in range(W):
                    TAj, TBj = TA0[j], TB0[j]
                    T = TAj + TBj
                    maggT_sb = spool.tile([H, 128], F32, tag="sg_maggT_sb")
                    if T > 0:
                        pay = gather_window(spool, "sg_pay", e0_idxA_in,
                                            e0_idxB_in, h2_full[:], h2B,
                                            offA, offB, TAj, TBj, H, Tmax0)
                        pay3 = pay[:].rearrange("p (t f) -> p t f", f=H)
                        G = build_G(spool, "sg_G", e0_lcol_sb, None, offT, T)
                        G3 = G[:].rearrange("p (t c) -> p t c", c=128)
                        maggT = sps.tile([H, 128], F32, tag="sg_maggT")
                        for t in range(T):
                            nc.tensor.matmul(maggT[:], pay3[:, t], G3[:, t],
                                             start=(t == 0), stop=(t == T - 1))
                        nc.vector.tensor_copy(maggT_sb[:], maggT[:])
                    else:
                        nc.vector.memset(maggT_sb[:], 0.0)
                    ml_ps = sps2.tile([128, H], F32, tag="sg_ml")
                    nc.tensor.matmul(ml_ps[:], maggT_sb[:], Wl_sb[:],
                                     start=True, stop=True)
                    wr_ps = sps2.tile([128, H], F32, tag="sg_wr")
                    nc.tensor.matmul(wr_ps[:], h2T_own[:, j * 128:(j + 1) * 128],
                                     Wr_sb[:], start=True, stop=True)
                    emb_sb = spool.tile([128, H], F32, tag="sg_emb")
                    nc.vector.tensor_scalar(emb_sb[:], ml_ps[:],
                                            inv0o_sb[:, j:j + 1], None, AOT.mult)
                    nc.vector.tensor_tensor(emb_sb[:], emb_sb[:], wr_ps[:], AOT.add)
                    nc.vector.tensor_tensor(emb_sb[:], emb_sb[:], blr_sb[:], AOT.add)
                    nc.sync.dma_start(
                        emb_out[:].rearrange("(w p) f -> w p f", p=128)[j],
                        emb_sb[:])
                    embT_ps = sps2.tile([H, 128], F32, tag="sg_embT")
                    nc.tensor.transpose(embT_ps[:], emb_sb[:], ident[:])
                    embT_sb = spool.tile([H, 128], F32, tag="sg_embT_sb")
                    nc.vector.tensor_copy(embT_sb[:], embT_ps[:])
                    hid_ps = sps2.tile([128, 64], F32, tag="sg_hid")
                    nc.tensor.matmul(hid_ps[:], embT_sb[:], ar1w_sb[:],
                                     start=True, stop=True)
                    hid_sb = spool.tile([128, 64], F32, tag="sg_hid_sb")
                    nc.vector.tensor_tensor(hid_sb[:], hid_ps[:], ar1br_sb[:],
                                            AOT.add)
                    nc.vector.tensor_scalar_max(hid_sb[:], hid_sb[:], 0.0)
                    hidT_ps = sps2.tile([64, 128], F32, tag="sg_hidT")
                    nc.tensor.transpose(hidT_ps[:], hid_sb[:], ident[:])
                    hidT_sb = spool.tile([64, 128], F32, tag="sg_hidT_sb")
                    nc.vector.tensor_copy(hidT_sb[:], hidT_ps[:])
                    o_ps = sps2.tile([128, 2], F32, tag="sg_o")
                    nc.tensor.matmul(o_ps[:], hidT_sb[:], ar2w_sb[:],
                                     start=True, stop=True)
                    o_sb = spool.tile([128, 2], F32, tag="sg_o_sb")
                    nc.vector.tensor_tensor(o_sb[:], o_ps[:], ar2br_sb[:], AOT.add)
                    nc.scalar.activation(o_sb[:], o_sb[:], ACTF.Sigmoid)
                    nc.sync.dma_start(
                        outp_out[:].rearrange("(w p) f -> w p f", p=128)[j],
                        o_sb[:])
                    offA += TAj
                    offB += TBj
                    offT += T

            if "E" not in phases:
                with tc.tile_pool(name="dummy_sb", bufs=1) as dpool:
                    z64 = dpool.tile([128, H], F32, tag="z64")
                    nc.vector.memset(z64[:], 0.0)
                    z2 = dpool.tile([128, 2], F32, tag="z2")
                    nc.vector.memset(z2[:], 0.0)
                    for j in range(W):
                        nc.sync.dma_start(
                            emb_out[:].rearrange("(w p) f -> w p f", p=128)[j], z64[:])
                        nc.sync.dma_start(
                            outp_out[:].rearrange("(w p) f -> w p f", p=128)[j], z2[:])

    if not nc.is_finalized():
        nc.finalize()
    return nc


# ----------------------------------------------------------------------------
# Top-level kernel
# ----------------------------------------------------------------------------

def _prep_inputs(x, edge_index, W1, b1, Wg, bg, att_src, att_dst, Wl, bl, Wr,
                 a1_w, a1_b, a2_w, a2_b, r1_w, r1_b, r2_w, r2_b,
                 n_cores=NCORES):
    x = np.ascontiguousarray(np.asarray(x, np.float32))
    N, F_IN = x.shape
    H = np.asarray(W1).shape[1]
    HEADS = np.asarray(att_src).shape[0]
    npc = -(-N // (n_cores * P)) * P
    W = npc // P
    N_pad = npc * n_cores

    row0 = np.asarray(edge_index[0]).astype(np.int64)
    col0 = np.asarray(edge_index[1]).astype(np.int64)
    loops = np.arange(N, dtype=np.int64)
    colL = np.concatenate([col0, loops])
    rowL = np.concatenate([row0, loops])
    deg = np.bincount(colL, minlength=N_pad).astype(np.float64)
    dinv = np.where(deg > 0, deg ** -0.5, 0.0).astype(np.float32)
    deg0 = np.bincount(col0, minlength=N_pad).astype(np.float64)
    inv0 = (1.0 / np.maximum(deg0, 1.0)).astype(np.float32)

    e1 = _build_edge_set(rowL, colL, dinv[rowL], W, npc, N)
    e0 = _build_edge_set(row0, col0, None, W, npc, N)

    Wg_ = np.ascontiguousarray(np.asarray(Wg, np.float32))
    att_s = np.asarray(att_src, np.float32)
    att_d = np.asarray(att_dst, np.float32)
    Wg3 = Wg_.reshape(H, HEADS, H)
    Wsrc = np.ascontiguousarray(np.einsum("khc,hc->kh", Wg3, att_s).astype(np.float32))
    Wdst = np.ascontiguousarray(np.einsum("khc,hc->kh", Wg3, att_d).astype(np.float32))
    ar1w = np.ascontiguousarray(np.concatenate(
        [np.asarray(a1_w, np.float32), np.asarray(r1_w, np.float32)], axis=1))
    ar2w = np.zeros((64, 2), np.float32)
    ar2w[0:32, 0] = np.asarray(a2_w, np.float32)[:, 0]
    ar2w[32:64, 1] = np.asarray(r2_w, np.float32)[:, 0]
    rep = lambda v: np.ascontiguousarray(
        np.tile(np.asarray(v, np.float32)[None, :], (128, 1)))
    ar1b = np.concatenate([np.asarray(a1_b, np.float32),
                           np.asarray(r1_b, np.float32)])
    ar2b = np.array([float(np.asarray(a2_b).reshape(-1)[0]),
                     float(np.asarray(r2_b).reshape(-1)[0])], np.float32)

    shared = dict(
        x=x,
        W1=np.ascontiguousarray(np.asarray(W1, np.float32)),
        Wg=Wg_, Wsrc=Wsrc, Wdst=Wdst,
        Wl=np.ascontiguousarray(np.asarray(Wl, np.float32)),
        Wr=np.ascontiguousarray(np.asarray(Wr, np.float32)),
        ar1w=ar1w, ar2w=ar2w,
        b1r=rep(b1), bgr=rep(bg), blr=rep(bl),
        ar1br=rep(ar1b), ar2br=rep(ar2b),
    )
    in_maps = []
    for c in range(n_cores):
        m = dict(shared)
        m["dinv_own"] = np.ascontiguousarray(
            dinv[c * npc:(c + 1) * npc].reshape(W, P).T)
        m["inv0_own"] = np.ascontiguousarray(
            inv0[c * npc:(c + 1) * npc].reshape(W, P).T)
        pc1, pc0 = e1["per_core"][c], e0["per_core"][c]
        m["e1_idxA"] = pc1["idxA"]
        m["e1_idxB"] = pc1["idxB"]
        m["e1_col"] = pc1["colidx"]
        m["e1_lcol"] = pc1["lcol"]
        m["e1_dinw"] = pc1["dinw"]
        m["e0_idxA"] = pc0["idxA"]
        m["e0_idxB"] = pc0["idxB"]
        m["e0_lcol"] = pc0["lcol"] if pc0["lcol"].shape[1] else np.zeros((128, 1), np.float32)
        in_maps.append(m)

    dims = dict(N=N, F_IN=F_IN, H=H, HEADS=HEADS, W=W, npc=npc)
    return dims, e1, e0, in_maps


_program_cache = {}


def kernel(x, edge_index, W1, b1, Wg, bg, att_src, att_dst, Wl, bl, Wr,
           a1_w, a1_b, a2_w, a2_b, r1_w, r1_b, r2_w, r2_b):
    x = np.asarray(x)
    N = x.shape[0]
    dims, e1, e0, in_maps = _prep_inputs(
        x, edge_index, W1, b1, Wg, bg, att_src, att_dst, Wl, bl, Wr,
        a1_w, a1_b, a2_w, a2_b, r1_w, r1_b, r2_w, r2_b)
    key = (dims["N"], dims["F_IN"], dims["H"], dims["HEADS"],
           tuple(e1["TA"]), tuple(e1["TB"]), tuple(e0["TA"]), tuple(e0["TB"]))
    if key not in _program_cache:
        _program_cache[key] = build_program(
            dims["N"], dims["F_IN"], dims["H"], dims["HEADS"],
            dims["W"], dims["npc"], e1, e0)
    nc = _program_cache[key]
    res = run_bass_kernel_spmd(nc, in_maps, list(range(NCORES)))
    emb = np.concatenate([res.results[c]["emb_own"] for c in range(NCORES)],
                         axis=0)[:N]
    outp = np.concatenate([res.results[c]["outp_own"] for c in range(NCORES)],
                          axis=0)[:N]
    return emb, np.ascontiguousarray(outp[:, 0:1]), np.ascontiguousarray(outp[:, 1:2])
